# revision 24
# baseline (speedup 1.0000x reference)
"""MoE layer kernel for Trainium2, 8 NeuronCores.

Math: the reference computes
    logits = x @ gate_w.T + gate_b              [B,S,E]
    gates  = softmax(logits); top2 -> renorm    [B,S,2]
    expert_out = x @ expert_w[e].T + expert_b   [B,S,E,O]  (dense)
    out[b,o] = sum_{s,k} top_g[b,s,k] * expert_out[b,s,top_i[b,s,k],o]

Because the output sums over s and k, define dense routing weights
    w[b,s,e] = sum_k top_g[b,s,k] * 1[top_i[b,s,k]==e]
(top-2 of softmax renormalized == sigmoid of logit differences).
Then
    z[b,e,i]   = sum_s w[b,s,e] * x[b,s,i]          (tiny matmul)
    wsum[b,e]  = sum_s w[b,s,e]
    out[b,o]   = sum_e z[b,e,:] @ expert_w[e,o,:] + sum_e wsum[b,e]*expert_b[e,o]

~0.7 GFLOP instead of 274 GFLOP; memory-bound on reading x + expert_w once.

Distribution: core b handles batch b for gating + z; an AllToAll exchanges z
rows so core e holds z[:, e, :]; core e runs expert e's [1024,1024] matmul.
Host sums the 8 partial [B,O] outputs.

Precision strategy: fp32 matmul on TRN2 runs the PE at ~1/3 rate (2-pass
LOW_HIGH). All big matmuls here use two-term bf16 splitting instead:
a = ah + al (ah = bf16(a), al = bf16(a - ah)); a@b is computed with a
stacked stationary [ah|al] and two moving passes (bh, bl), accumulating
  top    = ah@bh + ah@bl
  bottom = al@bh + al@bl        (al@bl term ~2^-36, harmless)
then result = top + bottom on DVE. Error ~2^-18 relative — f32-quality,
so the top-k gate selection is safe. Host pre-splits and pre-transposes
x so no [128,128] PE transposes are needed on device.
"""

import sys

import numpy as np

for _p in ("/opt/trn_rl_repo",):
    if _p not in sys.path:
        sys.path.insert(0, _p)

import ml_dtypes
import concourse.bass as bass
import concourse.mybir as mybir
from concourse import bacc
import concourse.tile as tile
from concourse.masks import make_identity

F32 = mybir.dt.float32
BF = mybir.dt.bfloat16
BF_NP = ml_dtypes.bfloat16
P = 128          # partitions
NCORES = 8
B = 8            # batch (== NCORES, one batch per core in stage 1)
E = 8            # experts (== NCORES, one expert per core in stage 2)
I = 1024         # input features
O = 1024         # output features
IC = I // P      # 8 chunks of the contraction dim
XW = I + 1       # x width: 1024 features + ones column carrying wsum
BIG = 1.0e30
GW = 512         # gate token-group width (moving free dim)


def build_nc(T: int = 2048):
    """Build the per-core SPMD Bass module. T = tokens per core."""
    NT = T // P                    # token tiles of 128
    TG = (T + GW - 1) // GW        # gate token groups

    nc = bacc.Bacc(num_devices=NCORES)

    xh_d = nc.dram_tensor("xh", [T, XW], BF, kind="ExternalInput")
    xl_d = nc.dram_tensor("xl", [T, XW], BF, kind="ExternalInput")
    xth_d = nc.dram_tensor("xth", [I, T], BF, kind="ExternalInput")
    xtl_d = nc.dram_tensor("xtl", [I, T], BF, kind="ExternalInput")
    gct_d = nc.dram_tensor("gct", [I, 40], BF, kind="ExternalInput")
    gbc_d = nc.dram_tensor("gbc", [E, 1], F32, kind="ExternalInput")
    wth_d = nc.dram_tensor("wth", [I, O], BF, kind="ExternalInput")
    wtl_d = nc.dram_tensor("wtl", [I, O], BF, kind="ExternalInput")
    ebr_d = nc.dram_tensor("ebr", [B, O], F32, kind="ExternalInput")
    out_d = nc.dram_tensor("out_p", [B, O], F32, kind="ExternalOutput")

    with tile.TileContext(nc) as tc:
        with (
            tc.tile_pool(name="singles", bufs=1) as singles,
            tc.tile_pool(name="dram", bufs=1, space="DRAM") as dram_pool,
        ):
            # ---- constants ----
            rank = nc.sync.partition_id()
            ident = singles.tile([E, E], F32)
            make_identity(nc, ident)
            gct_sb = singles.tile([P, IC, 40], BF)  # [bf16(gwT) |0| resid] stacked
            nc.sync.dma_start(
                out=gct_sb, in_=gct_d[:].rearrange("(c p) g -> p c g", p=P)
            )
            gbc_sb = singles.tile([E, 1], F32)  # gate_b as per-partition column
            nc.sync.dma_start(out=gbc_sb, in_=gbc_d[:])

            # tiny warm-up AllGather: wakes ncfw + absorbs the collective
            # entry cost while the kernel is still DMA-bound
            warm_in = dram_pool.tile([E, 4], F32)
            warm_out = dram_pool.tile([NCORES * E, 4], F32)
            warm_sb = singles.tile([E, 4], F32)
            nc.vector.memset(warm_sb, 0.0)
            nc.sync.dma_start(out=warm_in, in_=warm_sb)
            nc.gpsimd.collective_compute(
                "AllGather",
                mybir.AluOpType.bypass,
                replica_groups=[list(range(NCORES))],
                ins=[warm_in[:].opt()],
                outs=[warm_out[:].opt()],
            )

            L_sb = singles.tile([P, NT, E], F32)   # logits, [tok, e] layout
            l2_sb = singles.tile([E, T], F32)      # logits, [e, tok] layout

            # ---- phase 1: gate logitsT[e, tok] via split-bf16 matmuls ----
            with (
                tc.tile_pool(name="ps_lgt", bufs=1, space="PSUM") as ps_lgt,
                tc.tile_pool(name="sb_xt", bufs=4) as sb_xt,
            ):
                lgt = ps_lgt.tile([40, T], F32)
                for c in range(IC):
                    xth_c = sb_xt.tile([P, T], BF, tag="xth")
                    nc.sync.dma_start(out=xth_c, in_=xth_d[c * P : (c + 1) * P, :])
                    xtl_c = sb_xt.tile([P, T], BF, tag="xtl")
                    nc.sync.dma_start(out=xtl_c, in_=xtl_d[c * P : (c + 1) * P, :])
                    for pi, plane in enumerate((xth_c, xtl_c)):
                        for g in range(TG):
                            gs = slice(g * GW, min((g + 1) * GW, T))
                            nc.tensor.matmul(
                                lgt[:, gs],
                                gct_sb[:, c, :],
                                plane[:, gs],
                                start=(c == 0 and pi == 0),
                                stop=(c == IC - 1 and pi == 1),
                            )
                # evacuate: logitsT = top + bottom + gate_b (halves, so the
                # logit transposes can start while the 2nd half evacuates)
                for h in range(2):
                    hs = slice(h * (T // 2), (h + 1) * (T // 2))
                    nc.vector.tensor_scalar(
                        l2_sb[:, hs], lgt[0:E, hs], gbc_sb, None,
                        mybir.AluOpType.add,
                    )
                    nc.vector.tensor_add(
                        l2_sb[:, hs], l2_sb[:, hs], lgt[32 : 32 + E, hs]
                    )

            # ---- x planes resident in SBUF (loaded during gate compute) ----
            xh_sb = singles.tile([P, NT, XW], BF)
            xl_sb = singles.tile([P, NT, XW], BF)
            xhv = xh_d[:].rearrange("(t p) i -> p t i", p=P)
            xlv = xl_d[:].rearrange("(t p) i -> p t i", p=P)
            for t in range(NT):
                nc.sync.dma_start(out=xh_sb[:, t, :], in_=xhv[:, t, :])
                nc.sync.dma_start(out=xl_sb[:, t, :], in_=xlv[:, t, :])
            # expert weights last — only needed after the AllToAll
            wth_sb = singles.tile([P, IC, O], BF)
            wtl_sb = singles.tile([P, IC, O], BF)
            for c in range(IC):
                nc.sync.dma_start(
                    out=wth_sb[:, c, :], in_=wth_d[c * P : (c + 1) * P, :]
                )
                nc.sync.dma_start(
                    out=wtl_sb[:, c, :], in_=wtl_d[c * P : (c + 1) * P, :]
                )
            ebr_sb = singles.tile([B, O], F32)  # expert_b replicated over B rows
            nc.sync.dma_start(out=ebr_sb, in_=ebr_d[:])

            # transpose logits [8, T] -> [tok, e] tiles
            with tc.tile_pool(name="ps_lt", bufs=2, space="PSUM") as ps_lt:
                for t in range(NT):
                    ltp = ps_lt.tile([P, E], F32)
                    nc.tensor.transpose(
                        ltp, l2_sb[:, t * P : (t + 1) * P], ident
                    )
                    nc.vector.tensor_copy(L_sb[:, t, :], ltp)

            # ---- phase 2: gating math (top-2 softmax -> dense weights) ----
            gp = singles
            m1 = gp.tile([P, NT], F32)
            nc.vector.reduce_max(m1, L_sb, axis=mybir.AxisListType.X)
            is1 = gp.tile([P, NT, E], F32)
            nc.vector.tensor_tensor(
                is1,
                L_sb,
                m1[:, :, None].to_broadcast((P, NT, E)),
                mybir.AluOpType.is_ge,
            )
            lm = gp.tile([P, NT, E], F32)
            nc.vector.tensor_scalar(lm, is1, BIG, None, mybir.AluOpType.mult)
            nc.vector.tensor_sub(lm, L_sb, lm)  # masked logits (top-1 removed)
            m2 = gp.tile([P, NT], F32)
            nc.vector.reduce_max(m2, lm, axis=mybir.AxisListType.X)
            is2 = gp.tile([P, NT, E], F32)
            nc.vector.tensor_tensor(
                is2,
                lm,
                m2[:, :, None].to_broadcast((P, NT, E)),
                mybir.AluOpType.is_ge,
            )
            d12 = gp.tile([P, NT], F32)
            nc.vector.tensor_sub(d12, m2, m1)
            w2 = gp.tile([P, NT], F32)
            nc.scalar.activation(w2, d12, mybir.ActivationFunctionType.Sigmoid)
            w1 = gp.tile([P, NT], F32)
            nc.vector.tensor_scalar(
                w1, w2, -1.0, 1.0, mybir.AluOpType.mult, mybir.AluOpType.add
            )
            wd = gp.tile([P, NT, E], F32)
            nc.vector.tensor_tensor(
                wd, is1, w1[:, :, None].to_broadcast((P, NT, E)), mybir.AluOpType.mult
            )
            nc.vector.tensor_tensor(
                is2, is2, w2[:, :, None].to_broadcast((P, NT, E)), mybir.AluOpType.mult
            )
            nc.vector.tensor_add(wd, wd, is2)
            # split routing weights: wc = [bf16(wd) | bf16(wd - bf16(wd))]
            wc_sb = gp.tile([P, NT, 40], BF)
            nc.vector.memset(wc_sb, 0.0)
            nc.vector.tensor_copy(wc_sb[:, :, 0:E], wd)
            nc.vector.tensor_tensor(
                wc_sb[:, :, 32:40], wd, wc_sb[:, :, 0:E], mybir.AluOpType.subtract
            )

            # ---- phase 3: z[e, i] = sum_tok w[tok,e] * x[tok,i] (+ wsum col) ----
            z_sb = singles.tile([E, XW], F32)
            zsl = [slice(0, 512), slice(512, 1024), slice(1024, 1025)]
            if T < 512:
                zsl = [slice(0, 512), slice(512, 1024), slice(1024, 1025)]
            with tc.tile_pool(name="ps_z", bufs=1, space="PSUM") as ps_z:
                z_ps = ps_z.tile([40, XW], F32)
                for t in range(NT):
                    for pi, plane in enumerate((xh_sb, xl_sb)):
                        st = t == 0 and pi == 0
                        sp = t == NT - 1 and pi == 1
                        for s in zsl:
                            nc.tensor.matmul(
                                z_ps[:, s],
                                wc_sb[:, t, :],
                                plane[:, t, s],
                                start=st,
                                stop=sp,
                            )
                nc.vector.tensor_copy(z_sb, z_ps[0:E, :])
                nc.vector.tensor_add(z_sb, z_sb, z_ps[32 : 32 + E, :])

            # ---- phase 4: AllGather z; each core keeps rows for its expert ----
            z_dram = dram_pool.tile([E, XW], F32)
            zg_dram = dram_pool.tile([NCORES * E, XW], F32)
            nc.sync.dma_start(out=z_dram, in_=z_sb)
            nc.gpsimd.collective_compute(
                "AllGather",
                mybir.AluOpType.bypass,
                replica_groups=[list(range(NCORES))],
                ins=[z_dram[:].opt()],
                outs=[zg_dram[:].opt()],
            )
            # gathered rows are [b, e, :]; this core needs e == its rank
            zgv = zg_dram[:].rearrange("(b e) i -> b e i", e=E)
            zz_sb = singles.tile([B, XW], F32)
            nc.sync.dma_start(
                out=zz_sb[:, None, :], in_=zgv[:, bass.ds(rank, 1), :]
            )

            # ---- phase 5: out_p[b, o] = zz[b,:1024] @ wt + wsum[b]*expert_b ----
            zzT = singles.tile([P, IC, B], F32)
            zzc = singles.tile([P, IC, 40], BF)  # split-bf16 stacked stationary
            out_sb = singles.tile([B, O], F32)
            bias_sb = singles.tile([B, O], F32)
            with (
                tc.tile_pool(name="ps_zt", bufs=2, space="PSUM") as ps_zt,
                tc.tile_pool(name="ps_out", bufs=1, space="PSUM") as ps_out,
            ):
                for c in range(IC):
                    zt_ps = ps_zt.tile([P, B], F32)
                    nc.tensor.transpose(
                        zt_ps, zz_sb[:, c * P : (c + 1) * P], ident
                    )
                    nc.vector.tensor_copy(zzT[:, c, :], zt_ps)
                nc.vector.memset(zzc, 0.0)
                nc.vector.tensor_copy(zzc[:, :, 0:E], zzT)
                nc.vector.tensor_tensor(
                    zzc[:, :, 32:40], zzT, zzc[:, :, 0:E],
                    mybir.AluOpType.subtract,
                )

                out_ps = ps_out.tile([40, O], F32)
                for c in range(IC):
                    for pi, plane in enumerate((wth_sb, wtl_sb)):
                        st = c == 0 and pi == 0
                        sp = c == IC - 1 and pi == 1
                        nc.tensor.matmul(
                            out_ps[:, 0:512],
                            zzc[:, c, :],
                            plane[:, c, 0:512],
                            start=st,
                            stop=sp,
                        )
                        nc.tensor.matmul(
                            out_ps[:, 512:1024],
                            zzc[:, c, :],
                            plane[:, c, 512:1024],
                            start=st,
                            stop=sp,
                        )
                # bias: wsum[b] * expert_b[o], wsum is zz's last column
                nc.vector.tensor_scalar(
                    bias_sb, ebr_sb, zz_sb[:, I : I + 1], None, mybir.AluOpType.mult
                )
                nc.vector.tensor_add(out_sb, out_ps[0:E, :], bias_sb)
                nc.vector.tensor_add(out_sb, out_sb, out_ps[32 : 32 + E, :])
            nc.sync.dma_start(out=out_d[:], in_=out_sb)

    nc.compile()
    return nc


_NC_CACHE: dict = {}


def _get_nc(T: int):
    if T not in _NC_CACHE:
        _NC_CACHE[T] = build_nc(T)
    return _NC_CACHE[T]


def _split_bf16(a: np.ndarray):
    """a -> (ah, al) bf16 planes with ah + al ~= a (error ~2^-17 rel)."""
    ah = a.astype(BF_NP)
    al = (a - ah.astype(np.float32)).astype(BF_NP)
    return ah, al


def make_in_maps(x, gate_w, gate_b, expert_w, expert_b):
    x = np.ascontiguousarray(np.asarray(x, dtype=np.float32))
    gw = np.asarray(gate_w, dtype=np.float32)
    gb = np.asarray(gate_b, dtype=np.float32)
    ew = np.asarray(expert_w, dtype=np.float32)
    eb = np.asarray(expert_b, dtype=np.float32)
    _, S, _ = x.shape

    gwt = np.ascontiguousarray(gw.T)              # [I, E]
    gh, gl = _split_bf16(gwt)
    gct = np.zeros((I, 40), BF_NP)
    gct[:, 0:E] = gh
    gct[:, 32:40] = gl
    gbc = np.ascontiguousarray(gb.reshape(E, 1))

    in_maps = []
    for c in range(NCORES):
        xc = x[c]                                  # [S, I]
        xch, xcl = _split_bf16(xc)
        xh = np.ones((S, XW), BF_NP)
        xh[:, :I] = xch
        xl = np.zeros((S, XW), BF_NP)
        xl[:, :I] = xcl
        xth = np.ascontiguousarray(xch.T)          # [I, S]
        xtl = np.ascontiguousarray(xcl.T)
        wt = np.ascontiguousarray(ew[c].T)         # [I, O]
        wth, wtl = _split_bf16(wt)
        in_maps.append(
            {
                "xh": xh,
                "xl": xl,
                "xth": xth,
                "xtl": xtl,
                "gct": gct,
                "gbc": gbc,
                "wth": np.ascontiguousarray(wth),
                "wtl": np.ascontiguousarray(wtl),
                "ebr": np.ascontiguousarray(
                    np.broadcast_to(eb[c].reshape(1, O), (B, O))
                ),
            }
        )
    return in_maps, S


def _run(inputs: dict, trace: bool = False):
    from concourse.bass_utils import run_bass_kernel_spmd

    in_maps, S = make_in_maps(
        inputs["x"],
        inputs["gate_w"],
        inputs["gate_b"],
        inputs["expert_w"],
        inputs["expert_b"],
    )
    nc = _get_nc(S)
    res = run_bass_kernel_spmd(
        nc, in_maps, core_ids=list(range(NCORES)), trace=trace
    )
    out = np.zeros((B, O), np.float64)
    for c in range(NCORES):
        out += res.results[c]["out_p"].astype(np.float64)
    return out.astype(np.float32), res


def kernel(**inputs) -> np.ndarray:
    out, _ = _run(inputs, trace=False)
    return out


def run_traced(**inputs):
    out, res = _run(inputs, trace=True)
    return out, res
